# revision 30
# baseline (speedup 1.0000x reference)
"""Bahdanau additive attention, data-parallel over batch on 8 TRN2 NeuronCores.

Single-stream design: enc is streamed from HBM ONCE, in transposed layout
only (64 MiB/core vs 128 MiB for the old dual-layout kernel), and the
weighted sum runs on DVE with a hand-authored 2x_1p custom uop.
HW exec: 287.7us (vs 490us dual-layout baseline).

Math (per batch row b):
    proj[a, s]  = sum_c U[a, c] * enc[s, c]         # PE, A-major
    th[a, s]    = tanh(proj[a, s] + db[b, a])       # ScalarE, bias fused
    w[*, s]     = exp(sum_a v[a] * th[a, s])        # PE vdot, ScalarE Exp
    c[c]        = (sum_s w[s] * enct[c, s]) / Z     # DVE 2x mult+reduce

Key points:
  - enct host layout [b, T, q, k, j] = enc[b, 512T+j, 128k+q]: partitions
    carry the c-dim -> PE contracts over c (proj) and DVE reduces over the
    free s-dim (weighted sum). No on-chip or second-stream transpose.
  - proj: lhsT = U^T chunks (stationary, reused), rhs = enct tiles.
    16 MM of N=512 per 512-s super-tile = the 8192-cycle PE floor; PE
    measures 99.8% dense at the warm ~216ns/MM roofline (~252us).
  - energy: lhsT = v replicated across all 128 M-columns -> the vdot
    matmul broadcasts energy to 128 partitions for free (M=128 costs the
    same N cycles as M=1); Exp's accum_out gives the softmax denominator
    for free.
  - weighted sum: custom-DVE TENSOR_TENSOR_REDUCE with an added 2x_1p uop
    program (1222ns per FD=2048 op, exactly 2x the stock 1x rate; the
    stock InstTensorTensorReduce ISA op crashes the device). The stock
    accumulator readback (DVE_READ_ACCUMULATOR2 reads block 7's a_flop)
    returns packed write-port bits in 2x mode, so a second custom op
    reads block 4's a_flop via NEXT_ALU_OUT_A, adds a seed (chaining
    group partials in place), and writes [P,1] f32. ws ops and reads
    share ONE scratch out tile so the WAR chain serializes
    ws->read->next-ws (a_flop integrity); stock copy/TT/TS/reduce/
    reciprocal never write a_flops so other interleavings are safe.
  - SDMA engines round-robin ALL queued transfers at packet granularity
    (everything queued at once lands at the same LATE time), so the
    pipeline-fill groups' loads are gated behind the previous group's
    completion via 1-element dummy DMAs that block the HWDGE rings'
    FIFO trigger queues. Small first groups (1-2 super-tiles) start PE
    by ~16us; small last groups shrink the trailing DVE chain.
  - output is written [b, q, k] (partition-major); host transposes to
    [b, (k q)] = [b, 1024] - avoids byte-scatter DMA descriptors.

Engine profile/core: PE 252us busy (bottleneck, 99.8% dense), DVE 191us,
DMA ~180us, ScalarE 136us; ramp 16.5us, DVE tail ~13us, teardown 4us.
"""

import sys

sys.path.insert(0, "/opt/trn_rl_repo")

import numpy as np
import ml_dtypes

import concourse.bass as bass
from concourse import bacc
import concourse.mybir as mybir
import concourse.tile as tile
from concourse.bass_utils import run_bass_kernel_spmd

# ---------------------------------------------------------------------------
# Custom-DVE machinery: a hand-authored 2x_1p uop program for the custom
# TENSOR_TENSOR_REDUCE op (fused bf16 multiply+reduce at 2 elem/cycle/lane),
# plus a custom 1-cycle accumulator-read op.
#
# The 2x ws program: per cycle two packed bf16 pairs (lo, hi) -> products
# p0, p1 written to WR0_LO/WR0_HI; a running fp32 sum (acc += p0 + p1)
# rides the ALU bypass chain through blocks 3..7 (a_flops latched). The
# C1 scale of the stock op is omitted -- call sites pass s1=1.0.
#
# The stock DVE_READ_ACCUMULATOR2 (auto-appended when accum_out is given)
# reads block 7's a_flop, which holds packed write-port bits in 2x mode.
# Instead accum_out is left off and the accumulator is read with a separate
# custom op via NEXT_ALU_OUT_A at block 3 (= block 4's a_flop, clean in
# both 1x and 2x, and untouched by stock copy/TT/TS/reduce/reciprocal
# programs which never enable a_flops). The read adds a CONST_0 seed (used
# to chain partials) and writes a [P, 1] fp32. It consumes a 1-element
# in0 slice of the ws op's out tile: that orders read-after-ws (RAW) and
# ws'-after-read (WAR, all ws ops share one scratch out tile), and gives
# CoreSim a place to find the value (the patched sim reference stashes the
# accumulator at flat position 0 of the ws out).
# ---------------------------------------------------------------------------
from concourse.dve_uop import (
    UopConfig, InpSel, OutSel, AluInp, DelayInp, Trigger, OutPath, AluOp,
    DveOpSpec,
)
from concourse.dve_spec import lower as _dve_lower, Spec as _Spec
from concourse.dve_spec import Src0 as _Src0, Src1 as _Src1
from concourse.dve_spec import C0 as _C0, C1 as _C1
from concourse.dve_ops import TENSOR_TENSOR_REDUCE, DveOp as _DveOp
import concourse.dve_ops as _dve_ops_mod
import concourse.bass_isa as _bass_isa

_ENABLE = 1


def _mk_2x_uop(seed):
    u = UopConfig()
    u.enable_input(InpSel.SRC_0, 0)       # lane 0 -> stage0 PREV_ALU_OUT
    u.enable_input(InpSel.SRC_1, 1)       # -> delay lane 0
    u.enable_input(InpSel.SRC_0_HI, 2)    # -> delay lane 1
    u.enable_input(InpSel.SRC_1_HI, 3)    # -> delay lane 2
    u.enable_input(InpSel.CONST_0, 4)     # -> delay lane 3 (accum seed)
    lanes = (0, 1, 2, 3)
    for st in range(8):
        dp = u.datapath_config[st]
        dp.pass_through_delay(*lanes)
        if st == 0:
            dp.enable_alu(AluOp.MULTIPLY, AluInp.PREV_ALU_OUT,
                          AluInp.PREV_DELAY_0)
        elif st == 1:
            dp.enable_alu(AluOp.MULTIPLY, AluInp.PREV_DELAY_1,
                          AluInp.PREV_DELAY_2)
            dp.enable_delay_from_src(DelayInp.PREV_ALU_OUT, 0)   # park p0
        elif st == 2:
            dp.enable_alu(AluOp.ADD, AluInp.PREV_DELAY_0,
                          AluInp.PREV_ALU_OUT)
            dp.enable_delay_from_src(DelayInp.PREV_ALU_OUT, 1)   # park p1
        elif st == 3:
            if seed:
                dp.enable_alu(AluOp.BYPASS, AluInp.PREV_DELAY_3,
                              AluInp.PREV_DELAY_3)
            else:
                dp.enable_alu(AluOp.ADD, AluInp.CURR_ALU_OUT,
                              AluInp.PREV_ALU_OUT)
        else:
            dp.pass_through_alu()
        if st >= 3:
            dp.alu_out_a_enable = _ENABLE
    u.accum_enabled = _ENABLE
    if seed:
        u.trigger = (Trigger.COUNT, Trigger.NONE, Trigger.NONE)
        u.next_uop = (1, 0, 0)
        u.repeat_count = 1
    else:
        u.out[OutPath.WR0_LO] = OutSel.DELAY_0
        u.out[OutPath.WR0_HI] = OutSel.DELAY_1
        u.out_enable[OutPath.WR0_LO] = _ENABLE
        u.out_enable[OutPath.WR0_HI] = _ENABLE
        u.require_inp0 = _ENABLE
        u.require_inp1 = _ENABLE
        u.trigger = (Trigger.SRC_TENSOR_DONE, Trigger.NONE, Trigger.NONE)
        u.next_uop = (0, 0, 0)
    return u


def _mk_read_uop():
    u = UopConfig()
    u.enable_input(InpSel.SRC_0, 0)       # consumed (1 element), unused
    u.enable_input(InpSel.CONST_0, 1)     # seed -> delay lane 0
    for st in range(8):
        dp = u.datapath_config[st]
        dp.pass_through_delay(0)
        if st == 3:
            dp.enable_alu(AluOp.BYPASS, AluInp.NEXT_ALU_OUT_A,
                          AluInp.NEXT_ALU_OUT_A)
        elif st == 4:
            dp.enable_alu(AluOp.ADD, AluInp.PREV_ALU_OUT,
                          AluInp.PREV_DELAY_0)
        elif st > 4:
            dp.pass_through_alu()
    u.out_enable[OutPath.WR0_LO] = _ENABLE
    u.require_inp0 = _ENABLE
    u.repeat_count = 1
    u.trigger = (Trigger.COUNT, Trigger.NONE, Trigger.NONE)
    u.next_uop = (0, 0, 0)
    return u


def _ws_reference(in0, in1, c0, c1, c2):
    b = (in0.astype(np.float32) * in1 * c1).astype(np.float32)
    acc = c0 + b.reshape(b.shape[0], -1).sum(-1, keepdims=True)
    out = b.reshape(b.shape[0], -1).copy()
    out[:, 0:1] = acc
    return out.reshape(b.shape), acc


def _read_reference(in0, in1, c0, c1, c2):
    s = in0.astype(np.float32).reshape(in0.shape[0], -1)[:, 0:1]
    return s + c0


_ACC_READ_NAME = "TTR_ACC_READ_NEXT_ANT"


def _install_dve_ops(ver="v3"):
    import concourse.bass_utils as bu
    from operator import add as _add

    if (TENSOR_TENSOR_REDUCE.name, ver) in _dve_ops_mod._COMPILE_CACHE \
            and _ACC_READ_NAME in _dve_ops_mod._SUB_OPCODE_FOR_NAME:
        return
    from concourse.dve_ops import get_dve_sub_opcode

    ws = DveOpSpec(
        name=TENSOR_TENSOR_REDUCE.name,
        opcode=get_dve_sub_opcode(TENSOR_TENSOR_REDUCE.name),
        uops=_dve_lower(TENSOR_TENSOR_REDUCE.spec, ver=ver),
        uops_2x=[_mk_2x_uop(True), _mk_2x_uop(False)],
        rd1_en=True,
        perf_max=1,
    )
    ws.validate(ver)
    _dve_ops_mod._COMPILE_CACHE[(TENSOR_TENSOR_REDUCE.name, ver)] = ws
    _dve_ops_mod.CUSTOM_DVE_SPECS[TENSOR_TENSOR_REDUCE.name] = _Spec(
        body=_Src0 * _Src1 * _C1, accum=_add, accum_init=_C0,
        reference=_ws_reference)

    rd_op = _DveOp(_ACC_READ_NAME, _Spec(body=_Src0 + _C0,
                                         reference=_read_reference),
                   subdim=False, uops_sha={})
    _dve_ops_mod.OPS.append(rd_op)
    _dve_ops_mod.CUSTOM_DVE_SPECS[_ACC_READ_NAME] = rd_op.spec
    row = _dve_ops_mod._CUSTOM_DVE_ROW_BASE + len(_dve_ops_mod.OPS) - 1
    assert row < 0x20
    _dve_ops_mod._SUB_OPCODE_FOR_NAME[_ACC_READ_NAME] = row
    rd = DveOpSpec(name=_ACC_READ_NAME, opcode=row, uops=[_mk_read_uop()])
    rd.validate(ver)
    _dve_ops_mod._COMPILE_CACHE[(_ACC_READ_NAME, ver)] = rd
    try:
        bu._table_cache.clear()
    except AttributeError:
        pass


def _emit_ws2x(vec, *, out, in0, in1):
    ins = vec._custom_dve(
        TENSOR_TENSOR_REDUCE, out=out, in0=in0, in1=in1, s0=0.0, s1=1.0)
    ins.ins.perf_max = 1
    return ins


def _emit_acc_read(vec, *, out, dep, s0=0.0):
    from concourse.dve_ops import get_dve_sub_opcode

    bass_ = vec.bass
    if _ACC_READ_NAME not in bass_.m.ant_custom_dve_ops:
        bass_.m.ant_custom_dve_ops = sorted(
            {*bass_.m.ant_custom_dve_ops, _ACC_READ_NAME})
    shape = _bass_isa.CustomDveShape.TTSS
    isa_opcode = bass_.isa.Opcode[
        f"NEURON_ISA_TPB_OPCODE_CUSTOM_DVE_ANT_{shape.slot()}"
    ].value

    def lower_scalar(v):
        if isinstance(v, (int, float)):
            return mybir.ImmediateValue(dtype=mybir.dt.float32, value=float(v))
        return vec.lower_ap(v, for_isa=True)

    return vec.add_instruction(
        _bass_isa.InstCustomDveAnt(
            name=bass_.get_next_instruction_name(),
            op_name=_ACC_READ_NAME,
            rd1_en=False,
            subdim=0,
            imm2=0.0,
            shape=shape,
            row=get_dve_sub_opcode(_ACC_READ_NAME),
            isa_opcode=isa_opcode,
            ins=[vec.lower_ap(dep, for_isa=True),
                 lower_scalar(s0), lower_scalar(0.0)],
            outs=[vec.lower_ap(out, for_isa=True)],
        )
    )


_install_dve_ops()

B, S, A, DD, CTX = 64, 4096, 256, 1024, 1024
NCORES = 8
BL = B // NCORES       # 8 batches per core
P = 128
ST = 512               # s-rows per super-tile
NSUP = S // ST         # 8 super-tiles per batch
KC = CTX // P          # 8 c-chunks
NH = A // P            # 2 A-halves
HBS = 4                # super-tiles per DVE group (FD = HBS*ST = 2048)
NGRP = NSUP // HBS     # 2 groups per batch
NG = BL * NGRP         # 16 groups per core
BF16 = mybir.dt.bfloat16
F32 = mybir.dt.float32

# Per-batch group plans (t_base, n_super_tiles). The SDMA engines
# round-robin across ALL queued transfers at packet granularity, so
# group loads are GATED (serialized) behind the previous group's
# completion -- otherwise every queued chunk finishes at the same late
# time and the first projection starts ~10us late. Small first groups
# shorten the ramp; small last groups shorten the DVE ws tail.
_PLAN_FIRST = [(0, 1), (1, 1), (2, 2), (4, 2), (6, 2)]
_PLAN_MID = [(0, 4), (4, 4)]
_PLAN_LAST = [(0, 2), (2, 2), (4, 2), (6, 2)]
_N_GATES = 4  # gate only the pipeline-fill groups; steady state self-paces

_CACHE = {}


def _fast_bf16(x: np.ndarray) -> np.ndarray:
    """float32 -> bfloat16 (RNE)."""
    try:
        import jax, jax.numpy as jnp
        with jax.default_device(jax.devices("cpu")[0]):
            return np.asarray(jnp.asarray(x).astype(jnp.bfloat16))
    except Exception:
        u = np.ascontiguousarray(x, dtype=np.float32).view(np.uint32)
        r = ((u + 0x7FFF + ((u >> 16) & 1)) >> 16).astype(np.uint16)
        return r.view(ml_dtypes.bfloat16)


def _build():
    nc = bacc.Bacc()
    enct = nc.declare_dram_parameter("enct", [BL, NSUP, P, KC, ST], BF16,
                                     isOutput=False)
    ut = nc.declare_dram_parameter("ut", [P, KC, NH, P], BF16, isOutput=False)
    vrep = nc.declare_dram_parameter("vrep", [P, NH, P], BF16, isOutput=False)
    dbp = nc.declare_dram_parameter("dbp", [P, BL * NH], F32, isOutput=False)
    out = nc.declare_dram_parameter("out", [BL, P, KC], F32, isOutput=True)
    junk = nc.declare_dram_parameter("junk", [1, 64], BF16, isOutput=True)

    with tile.TileContext(nc) as tc:
        with (
            tc.tile_pool(name="const", bufs=1) as const,
            tc.tile_pool(name="trhp", bufs=4) as trhp,
            tc.tile_pool(name="wbcp", bufs=2) as wbcp,
            tc.tile_pool(name="thp", bufs=4) as thp,
            tc.tile_pool(name="accp", bufs=2) as accp,
            tc.tile_pool(name="epip", bufs=2) as epip,
            tc.tile_pool(name="psproj", bufs=4, space="PSUM") as psproj,
            tc.tile_pool(name="psw", bufs=2, space="PSUM") as psw,
            tc.tile_pool(name="psj", bufs=1, space="PSUM") as psj,
        ):
            # ---- constants ----
            ut_sb = const.tile([P, KC, NH, P], BF16)
            nc.sync.dma_start(ut_sb[:], ut[:])
            v_sb = const.tile([P, NH, P], BF16)
            nc.sync.dma_start(v_sb[:], vrep[:])
            db_sb = const.tile([P, BL * NH], F32)
            nc.sync.dma_start(db_sb[:], dbp[:])
            # ONE shared ws scratch tile: every ws op writes it and every
            # acc-read consumes a byte of it, so reads interleave strictly
            # between ws ops (a_flop integrity). ST+8 pad keeps APs
            # un-coalescible (shape-consistent custom-op operands).
            scr = const.tile([P, HBS, ST + 8], BF16)

            # flat group schedule: (b, t_base, nt, first_in_batch,
            # last_in_batch)
            plan = []
            for b in range(BL):
                pb = (_PLAN_FIRST if b == 0 else
                      _PLAN_LAST if b == BL - 1 else _PLAN_MID)
                for i, (t0_, nt_) in enumerate(pb):
                    plan.append((b, t0_, nt_, i == 0, i == len(pb) - 1))
            ng = len(plan)

            trh = {}    # g -> [P, HBS, KC, ST] bf16
            wbc = {}    # g -> [P, HBS, ST+8] bf16 (broadcast exp weights)
            zpart = {}  # b -> [P, NSUP] f32 (per-super-tile denominators)
            cacc = {}   # b -> [P, KC] f32 (seed-chained ws accumulator)

            def issue_load(g):
                if g >= ng:
                    return
                b, t0_, nt_, _, _ = plan[g]
                if 0 < g <= _N_GATES:
                    # gate: serialize this group's transfers behind the
                    # previous group's completion (1-element reads block
                    # both HWDGE rings' trigger queues until the prior
                    # group's chunk semaphores fire)
                    pb_, pt0, pnt, _, _ = plan[g - 1]
                    gsrc = trh[g - 1][0:1, pnt - 1, 0, 0:1]
                    nc.sync.dma_start(junk[0:1, 2 * g:2 * g + 1], gsrc)
                    nc.scalar.dma_start(junk[0:1, 2 * g + 1:2 * g + 2], gsrc)
                trh[g] = trhp.tile([P, HBS, KC, ST], BF16, tag="trh",
                                   name=f"trh{g}")
                for t in range(nt_):
                    eng = nc.sync if t % 2 == 0 else nc.scalar
                    eng.dma_start(trh[g][:, t], enct[b, t0_ + t])

            issue_load(0)

            # HAM warm-up: ~9 junk matmuls on the resident U^T tile while
            # the first enc chunk is still in flight, so the PE clock gate
            # is already at 8/8 (2.4 GHz) when the real stream starts
            # (measured: first real MM 16.5us -> 14.3us).
            jp = psj.tile([P, ST], F32, tag="jp")
            for _ in range(9):
                nc.tensor.matmul(jp[:], ut_sb[:, 0, 0, :],
                                 ut_sb[:, 0:4, 0, :],
                                 start=True, stop=True)

            for g in range(ng):
                b, t0_, nt_, first_g, last_g = plan[g]
                issue_load(g + 1)
                if first_g:
                    zpart[b] = accp.tile([P, NSUP], F32, tag="zp",
                                         name=f"zp{b}")
                    cacc[b] = accp.tile([P, KC], F32, tag="ca",
                                        name=f"ca{b}")
                wbc[g] = wbcp.tile([P, HBS, ST + 8], BF16, tag="wbc",
                                   name=f"wbc{g}")
                for t in range(nt_):
                    # projection: U^T stationary, enct streams; A-major out
                    ps = [psproj.tile([P, ST], F32, tag="proj",
                                      name=f"pj{g}_{t}_{h}")
                          for h in range(NH)]
                    for k in range(KC):
                        for h in range(NH):
                            nc.tensor.matmul(ps[h][:], ut_sb[:, k, h, :],
                                             trh[g][:, t, k, :],
                                             start=(k == 0),
                                             stop=(k == KC - 1))
                    # tanh with fused decoder bias
                    th = thp.tile([P, NH, ST], BF16, tag="th")
                    for h in range(NH):
                        nc.scalar.activation(
                            th[:, h, :], ps[h][:],
                            mybir.ActivationFunctionType.Tanh,
                            bias=db_sb[:, b * NH + h:b * NH + h + 1])
                    # energy, broadcast to 128 partitions via replicated v
                    wp = psw.tile([P, ST], F32, tag="wps")
                    for h in range(NH):
                        nc.tensor.matmul(wp[:], v_sb[:, h, :], th[:, h, :],
                                         start=(h == 0), stop=(h == NH - 1))
                    # exp -> bf16 weights; accum_out = denominator partial
                    tg = t0_ + t
                    nc.scalar.activation(
                        wbc[g][:, t, 0:ST], wp[:],
                        mybir.ActivationFunctionType.Exp,
                        accum_out=zpart[b][:, tg:tg + 1])
                # weighted sum over s: 2x fused mult+reduce (custom uop) then
                # a 1-cycle accumulator read, seed-chained in place
                for k in range(KC):
                    _emit_ws2x(nc.vector, out=scr[:, 0:nt_, 0:ST],
                               in0=trh[g][:, 0:nt_, k, :],
                               in1=wbc[g][:, 0:nt_, 0:ST])
                    _emit_acc_read(
                        nc.vector, out=cacc[b][:, k:k + 1],
                        dep=scr[:, 0, 0:1],
                        s0=(0.0 if first_g else cacc[b][:, k:k + 1]))
                if last_g:
                    # epilogue: Z, 1/Z, scale, store
                    zs = epip.tile([P, 1], F32, tag="zs")
                    nc.vector.tensor_reduce(zs[:], zpart[b][:],
                                            axis=mybir.AxisListType.X,
                                            op=mybir.AluOpType.add)
                    rec = epip.tile([P, 1], F32, tag="rec")
                    nc.vector.reciprocal(rec[:], zs[:])
                    cf2 = epip.tile([P, KC], F32, tag="cf2")
                    nc.vector.tensor_scalar_mul(cf2[:], cacc[b][:], rec[:])
                    nc.sync.dma_start(out[b], cf2[:])

    if not nc.is_finalized():
        nc.finalize()
    return nc


def kernel(previous_decoder_hidden_state, encoder_final_hidden_layers,
           W, b_W, U, b_U, v):
    prev = np.asarray(previous_decoder_hidden_state, dtype=np.float32)
    enc = np.asarray(encoder_final_hidden_layers, dtype=np.float32)
    W = np.asarray(W, dtype=np.float32)
    b_W = np.asarray(b_W, dtype=np.float32)
    U = np.asarray(U, dtype=np.float32)
    b_U = np.asarray(b_U, dtype=np.float32)
    v = np.asarray(v, dtype=np.float32)

    if "nc" not in _CACHE:
        _CACHE["nc"] = _build()
    nc = _CACHE["nc"]

    # ---- host-side prep ----
    enc_bf = _fast_bf16(enc)                                  # [B, S, CTX]
    # enct[b, T, q, k, j] = enc[b, 512T + j, 128k + q]
    e = enc_bf.reshape(B, NSUP, ST, KC, P)                    # [b,T,j,k,q]
    enct = np.ascontiguousarray(e.transpose(0, 1, 4, 3, 2))   # [b,T,q,k,j]
    # ut[q, k, h, m] = U[128h + m, 128k + q]
    UT = np.ascontiguousarray(U.T)                            # [CTX, A]
    ut_host = np.ascontiguousarray(
        UT.reshape(KC, P, NH, P).transpose(1, 0, 2, 3)
    ).astype(ml_dtypes.bfloat16)
    # vrep[a, h, m] = v[128h + a]  (replicated over m)
    v_host = np.ascontiguousarray(
        np.broadcast_to(v.reshape(NH, P).T[:, :, None], (P, NH, P))
    ).astype(ml_dtypes.bfloat16)
    # db[b, a] = (W @ prev[b] + b_W + b_U)[a]; device wants [q=a%128, b*NH+h]
    db = prev @ W.T + b_W + b_U                               # [B, A] f32
    dbr = np.ascontiguousarray(
        db.reshape(B, NH, P).transpose(2, 0, 1).reshape(P, B * NH)
    ).astype(np.float32)

    in_maps = []
    for i in range(NCORES):
        sl = slice(i * BL, (i + 1) * BL)
        in_maps.append({
            "enct": enct[sl],
            "ut": ut_host,
            "vrep": v_host,
            "dbp": dbr[:, i * BL * NH:(i + 1) * BL * NH],
        })

    res = run_bass_kernel_spmd(nc, in_maps, list(range(NCORES)),
                               **_CACHE.get("run_kwargs", {}))
    _CACHE["last_result"] = res
    outs = [np.asarray(r["out"]) for r in res.results]        # [BL, P, KC]
    full = np.concatenate(outs, axis=0)                       # [B, P, KC]
    return np.ascontiguousarray(
        full.transpose(0, 2, 1).reshape(B, CTX)).astype(np.float32)
